# revision 5
# baseline (speedup 1.0000x reference)
"""Trainium2 Bass kernel for nn_Compute_all_u (embedding gather + batched affine dot).

For each voxel v:
    u[v, :] = C[e_v, 0, :] + x_v*C[e_v, 1, :] + y_v*C[e_v, 2, :] + z_v*C[e_v, 3, :]
where e_v = voxels_elements[v], (x,y,z) = all_voxels_centroids[v].

Strategy ("broadcast-R"): shard the ELEMENT TABLE across the 8 cores
(62,500 elements each) and route voxels to the core owning their element.
Each element is then referenced ~16x per core (Poisson(16)), so the device
never needs data-dependent addressing: the host sorts voxels by element and
packs each element's voxels into ceil(L/8) groups of R=8 consecutive slots;
the device streams one (host-repeated) table row per group and broadcasts it
across the group's 8 slots with stride-0 access patterns on outer axes.

This removes the SWDGE dma_gather entirely - the v1 kernel was bottlenecked
at ~8.7ns/row of Q7 descriptor generation (1M rows / 4 queues = 2.26ms),
with DMA engines only ~14% busy. Here everything is sequential DMA + math.

Layouts are PLANAR so every elementwise operand has innermost stride 1 (the
DVE 2x_1P fp16 perf mode requires step_x=+-1 / 4B alignment on all srcs and
dst; broadcasts live on outer axes where stride 0 is allowed):
  trow[t, p, dk, c]   dk = d*3+k       (12 planes of CG rows)
  cent[t, p, j, r, c] j in {x,y,z}     (3 planes of R x CG)
with group g = (t*128 + p)*CG + c holding slots s = g*R + r.

The 6 elementwise fp16 ops per (tile, r-range) are:
  tmp = X(bcast k) * C1(bcast r);  u  = C0(bcast r) + tmp
  tmp = Y(bcast k) * C2(bcast r);  u += tmp
  tmp = Z(bcast k) * C3(bcast r);  u += tmp
r-slots 0:6 run on the DVE (~230 G elem/s in 2x mode), r-slots 6:8 on the
Pool/GpSimd engine (~67 G elem/s) so both engines compute concurrently,
each writing its own packed output stream (out_a / out_b).

Precision: fp16 throughout; measured rel err ~1e-3 vs the f32 reference
(gate 2e-2): values are O(1) normals, u ~ N(0, 4), fp16 eps 9.8e-4.

Host prep per call: one 8M argsort by element, per-core bincount/cumsum to
assign slots, np.repeat to build the group row stream (~2.4x the 3MB table
slice), scatter centroids into slot-planar order, un-permute outputs. Any
voxel whose slot would exceed the padded group capacity NG (the actual
seed-0 per-core max is 152,343 vs NG=153,600) falls back to exact host math.
"""

import numpy as np

from concourse import bacc, bass, tile, mybir
from concourse.bass_utils import run_bass_kernel_spmd

N_VOXELS = 8_000_000
N_ELEM = 500_000
N_CORES = 8
EPC = N_ELEM // N_CORES     # 62,500 elements per core
R = 8                       # slots per group (one broadcast row each)
RA = 6                      # r-slots computed by the DVE; the rest by Pool
RB = R - RA
CG = 120                    # groups per partition per tile (240B rows)
NT = 10                     # tiles per core
NG = NT * 128 * CG          # 153,600 group capacity (seed-0 max 152,343)
NSLOT = NG * R              # 1,228,800 slots per core

f16 = mybir.dt.float16


def build_nc(bufs: int = 6) -> bass.Bass:
    nc = bacc.Bacc("TRN2")
    trow_in = nc.declare_dram_parameter("trow", [NT, 128, 12 * CG], f16, isOutput=False)
    cent_in = nc.declare_dram_parameter("cent", [NT, 128, 3 * R * CG], f16, isOutput=False)
    out_a = nc.declare_dram_parameter("out_a", [NT, 128, 3 * RA * CG], f16, isOutput=True)
    out_b = nc.declare_dram_parameter("out_b", [NT, 128, 3 * RB * CG], f16, isOutput=True)

    mul = mybir.AluOpType.mult
    add = mybir.AluOpType.add

    with tile.TileContext(nc) as tc:
        with (
            tc.tile_pool(name="io", bufs=bufs) as io_pool,
            tc.tile_pool(name="tmp", bufs=2) as tmp_pool,
        ):
            for t in range(NT):
                trow_t = io_pool.tile([128, 12 * CG], f16, tag="trow")
                nc.sync.dma_start(out=trow_t[:], in_=trow_in[t])
                cent_t = io_pool.tile([128, 3 * R * CG], f16, tag="cent")
                nc.sync.dma_start(out=cent_t[:], in_=cent_in[t])

                tr = trow_t[:].rearrange("p (dk c) -> p dk c", c=CG)
                cr = cent_t[:].rearrange("p (j r c) -> p j r c", r=R, c=CG)

                for eng, r0, r1, dram in (
                    (nc.vector, 0, RA, out_a),
                    (nc.gpsimd, RA, R, out_b),
                ):
                    w = r1 - r0
                    u = io_pool.tile([128, 3 * w * CG], f16, tag=f"u{r0}")
                    tmp = tmp_pool.tile([128, 3 * w * CG], f16, tag=f"t{r0}")
                    ur = u[:].rearrange("p (k r c) -> p k r c", r=w, c=CG)
                    tmr = tmp[:].rearrange("p (k r c) -> p k r c", r=w, c=CG)

                    def rows(d):  # trow planes d*3..d*3+3, bcast over r (outer)
                        return tr[:, 3 * d:3 * d + 3, :].unsqueeze(2).to_broadcast(
                            [128, 3, w, CG]
                        )

                    def xyz(j):  # cent plane j, r-range slice, bcast over k
                        return cr[:, j:j + 1, r0:r1, :].to_broadcast([128, 3, w, CG])

                    eng.tensor_tensor(out=tmr, in0=xyz(0), in1=rows(1), op=mul)
                    eng.tensor_tensor(out=ur, in0=rows(0), in1=tmr, op=add)
                    eng.tensor_tensor(out=tmr, in0=xyz(1), in1=rows(2), op=mul)
                    eng.tensor_tensor(out=ur, in0=ur, in1=tmr, op=add)
                    eng.tensor_tensor(out=tmr, in0=xyz(2), in1=rows(3), op=mul)
                    eng.tensor_tensor(out=ur, in0=ur, in1=tmr, op=add)

                    nc.sync.dma_start(out=dram[t], in_=u[:])
    nc.finalize()
    return nc


_NC_CACHE: dict = {}


def _get_nc():
    key = (R, RA, CG, NT)
    if key not in _NC_CACHE:
        _NC_CACHE[key] = build_nc()
    return _NC_CACHE[key]


def _prep_core(el, vox, coeffs16_c, cent16_full):
    """Build one core's device arrays from its (sorted) local element ids."""
    n = el.shape[0]
    counts = np.bincount(el, minlength=EPC)
    ngrp = (counts + (R - 1)) // R
    gbase = np.zeros(EPC, dtype=np.int64)
    np.cumsum(ngrp[:-1], out=gbase[1:])
    run_start = np.zeros(EPC, dtype=np.int64)
    np.cumsum(counts[:-1], out=run_start[1:])
    rank = np.arange(n, dtype=np.int64) - run_start[el]
    slot = gbase[el] * R + rank
    ok = slot < NSLOT

    trow_flat = np.zeros((NG, 12), dtype=np.float16)
    total_g = int(ngrp.sum())
    if total_g <= NG:
        trow_flat[:total_g] = np.repeat(coeffs16_c, ngrp, axis=0)
    else:
        trow_flat[:] = np.repeat(coeffs16_c, ngrp, axis=0)[:NG]

    cent_slot = np.zeros((NSLOT, 3), dtype=np.float16)
    cent_slot[slot[ok]] = cent16_full[vox[ok]]

    # planar device layouts (innermost = group axis c)
    trow_dev = np.ascontiguousarray(
        trow_flat.reshape(NT, 128, CG, 12).transpose(0, 1, 3, 2)
    ).reshape(NT, 128, 12 * CG)
    cent_dev = np.ascontiguousarray(
        cent_slot.reshape(NT, 128, CG, R, 3).transpose(0, 1, 4, 3, 2)
    ).reshape(NT, 128, 3 * R * CG)

    return (
        {"trow": trow_dev, "cent": cent_dev},
        slot,
        ok,
    )


def kernel(all_coeffs, all_voxels_centroids, voxels_elements, _trace=False, **run_kwargs):
    nc = _get_nc()
    coeffs12 = np.asarray(all_coeffs, dtype=np.float32).reshape(N_ELEM, 12)
    coeffs16 = coeffs12.astype(np.float16)
    cent_full = np.asarray(all_voxels_centroids, dtype=np.float32)
    cent16 = cent_full.astype(np.float16)
    e_full = np.asarray(voxels_elements).astype(np.int64)

    order = np.argsort(e_full, kind="stable")
    es = e_full[order]
    bounds = np.searchsorted(es, np.arange(N_CORES + 1, dtype=np.int64) * EPC)

    in_maps, metas = [], []
    for c in range(N_CORES):
        lo, hi = int(bounds[c]), int(bounds[c + 1])
        vox = order[lo:hi]
        el = (es[lo:hi] - c * EPC).astype(np.int64)
        m, slot, ok = _prep_core(el, vox, coeffs16[c * EPC:(c + 1) * EPC], cent16)
        in_maps.append(m)
        metas.append((vox, slot, ok))

    res = run_bass_kernel_spmd(
        nc, in_maps, core_ids=list(range(N_CORES)), trace=_trace, **run_kwargs
    )

    full = np.empty((N_VOXELS, 3), dtype=np.float32)
    for c in range(N_CORES):
        vox, slot, ok = metas[c]
        ua = res.results[c]["out_a"].reshape(NT, 128, 3, RA, CG)
        ub = res.results[c]["out_b"].reshape(NT, 128, 3, RB, CG)
        u_slots = np.ascontiguousarray(
            np.concatenate([ua, ub], axis=3).transpose(0, 1, 4, 3, 2)
        ).reshape(NSLOT, 3)
        full[vox[ok]] = u_slots[slot[ok]].astype(np.float32)
        bad = ~ok
        if bad.any():
            vb = vox[bad]
            cf = coeffs12[e_full[vb]].reshape(-1, 4, 3)
            xyz = cent_full[vb]
            full[vb] = cf[:, 0] + np.einsum("nd,ndk->nk", xyz, cf[:, 1:4])
    if _trace:
        return full, res
    return full


# revision 6
# speedup vs baseline: 1.5266x; 1.5266x over previous
"""Trainium2 Bass kernel for nn_Compute_all_u (embedding gather + batched affine dot).

For each voxel v:
    u[v, :] = C[e_v, 0, :] + x_v*C[e_v, 1, :] + y_v*C[e_v, 2, :] + z_v*C[e_v, 3, :]
where e_v = voxels_elements[v], (x,y,z) = all_voxels_centroids[v].

Strategy ("broadcast-R"): shard the ELEMENT TABLE across the 8 cores
(62,500 elements each) and route voxels to the core owning their element.
Each element is then referenced ~16x per core (Poisson(16)), so the device
never needs data-dependent addressing: the host sorts voxels by element and
packs each element's voxels into ceil(L/8) groups of R=8 consecutive slots;
the device streams one (host-repeated) table row per group and broadcasts it
across the group's 8 slots with stride-0 access patterns on outer axes.

This removes the SWDGE dma_gather entirely - the v1 kernel was bottlenecked
at ~8.7ns/row of Q7 descriptor generation (1M rows / 4 queues = 2.26ms),
with DMA engines only ~14% busy. Here everything is sequential DMA + DVE.
(Offloading a slice to the Pool engine was tried and REGRESSED: co-running
Pool with DVE halves both engines' SBUF throughput - kept all-DVE.)

Layouts are PLANAR so every DVE operand has innermost stride 1 (the 2x_1P
fp16 perf mode requires step_x=+-1 / 4B alignment on all srcs and dst;
broadcasts live on outer axes where stride 0 is allowed):
  trow[t, p, dk, c]   dk = d*3+k       (12 planes of cg rows)
  cent[t, p, j, r, c] j in {x,y,z}     (3 planes of R x cg)
with group g mapped tile-major / partition / column, slots s = g*R + r.

Per tile the 6 fp16 DVE ops (out shape [128, 3, R, cg]) are:
  tmp = X(bcast k) * C1(bcast r);  u  = C0(bcast r) + tmp
  tmp = Y(bcast k) * C2(bcast r);  u += tmp
  tmp = Z(bcast k) * C3(bcast r);  u += tmp

Tiles are SIZE-GRADED (4x30 + 8x120 + 2x60 group-columns) so the first DVE
op only waits on a quarter-size DMA and the tail drains quickly.

Precision: fp16 throughout; measured rel err ~1e-3 vs the f32 reference
(gate 2e-2): values are O(1) normals, u ~ N(0, 4), fp16 eps 9.8e-4.

Host prep per call: one 8M argsort by element, per-core bincount/cumsum to
assign slots, np.repeat to build the group row stream (~2.4x the 3MB table
slice), scatter centroids into slot-planar order, un-permute outputs. Any
voxel whose slot would exceed the padded group capacity NG (the actual
seed-0 per-core max is 152,343 vs NG=153,600) falls back to exact host math.
"""

import numpy as np

from concourse import bacc, bass, tile, mybir
from concourse.bass_utils import run_bass_kernel_spmd

N_VOXELS = 8_000_000
N_ELEM = 500_000
N_CORES = 8
EPC = N_ELEM // N_CORES     # 62,500 elements per core
R = 8                       # slots per group (one broadcast row each)

# tile regions: (name, n_tiles, group-columns per partition per tile)
REGIONS = (("h", 4, 30), ("m", 8, 120), ("t", 2, 60))
NG = sum(n * 128 * cg for _, n, cg in REGIONS)   # 153,600 (seed-0 max 152,343)
NSLOT = NG * R                                   # 1,228,800 slots per core

f16 = mybir.dt.float16


def build_nc(bufs: int = 6) -> bass.Bass:
    nc = bacc.Bacc("TRN2")
    params = {}
    for name, n, cg in REGIONS:
        params[name] = (
            nc.declare_dram_parameter(f"trow_{name}", [n, 128, 12 * cg], f16, isOutput=False),
            nc.declare_dram_parameter(f"cent_{name}", [n, 128, 3 * R * cg], f16, isOutput=False),
            nc.declare_dram_parameter(f"out_{name}", [n, 128, 3 * R * cg], f16, isOutput=True),
            n,
            cg,
        )

    mul = mybir.AluOpType.mult
    add = mybir.AluOpType.add

    with tile.TileContext(nc) as tc:
        with (
            tc.tile_pool(name="io", bufs=bufs) as io_pool,
            tc.tile_pool(name="tmp", bufs=2) as tmp_pool,
        ):
            for name, n, cg in REGIONS:
                trow_in, cent_in, out, _, _ = params[name]
                for t in range(n):
                    trow_t = io_pool.tile([128, 12 * cg], f16, tag=f"trow{name}")
                    nc.sync.dma_start(out=trow_t[:], in_=trow_in[t])
                    cent_t = io_pool.tile([128, 3 * R * cg], f16, tag=f"cent{name}")
                    nc.sync.dma_start(out=cent_t[:], in_=cent_in[t])

                    u = io_pool.tile([128, 3 * R * cg], f16, tag=f"u{name}")
                    tmp = tmp_pool.tile([128, 3 * R * cg], f16, tag=f"t{name}")

                    tr = trow_t[:].rearrange("p (dk c) -> p dk c", c=cg)
                    cr = cent_t[:].rearrange("p (j r c) -> p j r c", r=R, c=cg)
                    ur = u[:].rearrange("p (k r c) -> p k r c", r=R, c=cg)
                    tmr = tmp[:].rearrange("p (k r c) -> p k r c", r=R, c=cg)

                    def rows(d):  # trow planes d*3..d*3+3, bcast over r
                        return tr[:, 3 * d:3 * d + 3, :].unsqueeze(2).to_broadcast(
                            [128, 3, R, cg]
                        )

                    def xyz(j):  # cent plane j, bcast over k
                        return cr[:, j:j + 1, :, :].to_broadcast([128, 3, R, cg])

                    nc.vector.tensor_tensor(out=tmr, in0=xyz(0), in1=rows(1), op=mul)
                    nc.vector.tensor_tensor(out=ur, in0=rows(0), in1=tmr, op=add)
                    nc.vector.tensor_tensor(out=tmr, in0=xyz(1), in1=rows(2), op=mul)
                    nc.vector.tensor_tensor(out=ur, in0=ur, in1=tmr, op=add)
                    nc.vector.tensor_tensor(out=tmr, in0=xyz(2), in1=rows(3), op=mul)
                    nc.vector.tensor_tensor(out=ur, in0=ur, in1=tmr, op=add)

                    nc.sync.dma_start(out=out[t], in_=u[:])
    nc.finalize()
    return nc


_NC_CACHE: dict = {}


def _get_nc():
    key = REGIONS
    if key not in _NC_CACHE:
        _NC_CACHE[key] = build_nc()
    return _NC_CACHE[key]


def _planar_regions(flat, inner):
    """Slice [NG, inner] group-major data into per-region planar arrays."""
    out = {}
    g0 = 0
    for name, n, cg in REGIONS:
        g1 = g0 + n * 128 * cg
        blk = flat[g0:g1].reshape(n, 128, cg, inner)
        # [n, p, c, inner] -> [n, p, inner, c] planar
        out[name] = np.ascontiguousarray(blk.transpose(0, 1, 3, 2)).reshape(
            n, 128, inner * cg
        )
        g0 = g1
    return out


def _prep_core(el, vox, coeffs16_c, cent16_full):
    """Build one core's device arrays from its (sorted) local element ids."""
    n = el.shape[0]
    counts = np.bincount(el, minlength=EPC)
    ngrp = (counts + (R - 1)) // R
    gbase = np.zeros(EPC, dtype=np.int64)
    np.cumsum(ngrp[:-1], out=gbase[1:])
    run_start = np.zeros(EPC, dtype=np.int64)
    np.cumsum(counts[:-1], out=run_start[1:])
    rank = np.arange(n, dtype=np.int64) - run_start[el]
    slot = gbase[el] * R + rank
    ok = slot < NSLOT

    trow_flat = np.zeros((NG, 12), dtype=np.float16)
    total_g = int(ngrp.sum())
    if total_g <= NG:
        trow_flat[:total_g] = np.repeat(coeffs16_c, ngrp, axis=0)
    else:
        trow_flat[:] = np.repeat(coeffs16_c, ngrp, axis=0)[:NG]

    cent_slot = np.zeros((NSLOT, 3), dtype=np.float16)
    cent_slot[slot[ok]] = cent16_full[vox[ok]]

    in_map = {}
    for name, arr in _planar_regions(trow_flat, 12).items():
        in_map[f"trow_{name}"] = arr
    # cent: [NSLOT, 3] -> group-major [NG, R*3] -> per-region planar with
    # inner axes (r, j) transposed to (j, r)
    cent_grp = cent_slot.reshape(NG, R, 3)
    g0 = 0
    for name, nt, cg in REGIONS:
        g1 = g0 + nt * 128 * cg
        blk = cent_grp[g0:g1].reshape(nt, 128, cg, R, 3)
        in_map[f"cent_{name}"] = np.ascontiguousarray(
            blk.transpose(0, 1, 4, 3, 2)
        ).reshape(nt, 128, 3 * R * cg)
        g0 = g1

    return in_map, slot, ok


def _reassemble(results_c):
    """Concatenate per-region outputs back to [NSLOT, 3] in slot order."""
    parts = []
    for name, nt, cg in REGIONS:
        blk = results_c[f"out_{name}"].reshape(nt, 128, 3, R, cg)
        parts.append(
            np.ascontiguousarray(blk.transpose(0, 1, 4, 3, 2)).reshape(-1, 3)
        )
    return np.concatenate(parts, axis=0)


def kernel(all_coeffs, all_voxels_centroids, voxels_elements, _trace=False, **run_kwargs):
    nc = _get_nc()
    coeffs12 = np.asarray(all_coeffs, dtype=np.float32).reshape(N_ELEM, 12)
    coeffs16 = coeffs12.astype(np.float16)
    cent_full = np.asarray(all_voxels_centroids, dtype=np.float32)
    cent16 = cent_full.astype(np.float16)
    e_full = np.asarray(voxels_elements).astype(np.int64)

    order = np.argsort(e_full, kind="stable")
    es = e_full[order]
    bounds = np.searchsorted(es, np.arange(N_CORES + 1, dtype=np.int64) * EPC)

    in_maps, metas = [], []
    for c in range(N_CORES):
        lo, hi = int(bounds[c]), int(bounds[c + 1])
        vox = order[lo:hi]
        el = (es[lo:hi] - c * EPC).astype(np.int64)
        m, slot, ok = _prep_core(el, vox, coeffs16[c * EPC:(c + 1) * EPC], cent16)
        in_maps.append(m)
        metas.append((vox, slot, ok))

    res = run_bass_kernel_spmd(
        nc, in_maps, core_ids=list(range(N_CORES)), trace=_trace, **run_kwargs
    )

    full = np.empty((N_VOXELS, 3), dtype=np.float32)
    for c in range(N_CORES):
        vox, slot, ok = metas[c]
        u_slots = _reassemble(res.results[c])
        full[vox[ok]] = u_slots[slot[ok]].astype(np.float32)
        bad = ~ok
        if bad.any():
            vb = vox[bad]
            cf = coeffs12[e_full[vb]].reshape(-1, 4, 3)
            xyz = cent_full[vb]
            full[vb] = cf[:, 0] + np.einsum("nd,ndk->nk", xyz, cf[:, 1:4])
    if _trace:
        return full, res
    return full


# revision 8
# speedup vs baseline: 1.6928x; 1.1089x over previous
"""Trainium2 Bass kernel for nn_Compute_all_u (embedding gather + batched affine dot).

For each voxel v:
    u[v, :] = C[e_v, 0, :] + x_v*C[e_v, 1, :] + y_v*C[e_v, 2, :] + z_v*C[e_v, 3, :]
where e_v = voxels_elements[v], (x,y,z) = all_voxels_centroids[v].

Strategy ("broadcast-R"): shard the ELEMENT TABLE across the 8 cores
(62,500 elements each) and route voxels to the core owning their element.
Each element is then referenced ~16x per core (Poisson(16)), so the device
never needs data-dependent addressing: the host sorts voxels by element and
packs each element's voxels into groups of consecutive slots that share one
(host-repeated) table row; the device streams rows + slot-ordered centroids
and broadcasts each row across its group with stride-0 access patterns.

This removes the SWDGE dma_gather entirely - the v1 kernel was bottlenecked
at ~8.7ns/row of Q7 descriptor generation (1M rows / 4 queues = 2.26ms),
with DMA engines only ~14% busy. Here everything is sequential DMA + DVE.
(Offloading a slice to the Pool engine was tried and REGRESSED: co-running
Pool with DVE halves both engines' SBUF throughput - kept all-DVE.)

MIXED GROUP SIZES cut slot padding: an element with count L gets
floor(L/8) full R=8 groups, plus (if the remainder m=L%8 is 5..7) one more
R=8 group, while small remainders m=1..4 go to a separate R=4 region.
Seed-0 slots: 1.11M vs 1.23M for uniform R=8 (~10% less DVE+DMA work).

Layouts are PLANAR so every DVE operand has innermost stride 1 (the 2x_1P
fp16 perf mode requires step_x=+-1 / 4B alignment on all srcs and dst;
broadcasts live on outer axes where stride 0 is allowed):
  trow[t, p, dk, c]   dk = d*3+k       (12 planes of cg rows)
  cent[t, p, j, r, c] j in {x,y,z}     (3 planes of Rreg x cg)
with group g mapped tile-major / partition / column, slots s = g*Rreg + r.

Per tile the 6 fp16 DVE ops (out shape [128, 3, Rreg, cg]) are:
  tmp = X(bcast k) * C1(bcast r);  u  = C0(bcast r) + tmp
  tmp = Y(bcast k) * C2(bcast r);  u += tmp
  tmp = Z(bcast k) * C3(bcast r);  u += tmp

Tiles are SIZE-GRADED (small head tiles) so the first DVE op only waits on
a quarter-size DMA; output stores issue from the Activation engine's HWDGE
queue so tile loads never queue behind them.

Precision: fp16 throughout; measured rel err ~1e-3 vs the f32 reference
(gate 2e-2): values are O(1) normals, u ~ N(0, 4), fp16 eps 9.8e-4.

Host prep per call: one 8M argsort by element, per-core bincount/cumsum to
assign slots, np.repeat to build the group row streams, scatter centroids
into slot-planar order, un-permute outputs. Any voxel whose slot would
exceed a region capacity (seed-0 actual: A 121,418/122,880; B 31,153/32,768)
falls back to exact host math.
"""

import numpy as np

from concourse import bacc, bass, tile, mybir
from concourse.bass_utils import run_bass_kernel_spmd

N_VOXELS = 8_000_000
N_ELEM = 500_000
N_CORES = 8
EPC = N_ELEM // N_CORES     # 62,500 elements per core
RA = 8                      # region-A slots per group
RB = 4                      # region-B slots per group (small remainders)

# device tile schedule: (region, n_tiles, group-columns per partition, R)
# A capacity 960 cols = 122,880 groups; B capacity 256 cols = 32,768 groups
TILES = (
    ("A", 4, 30, RA),       # small head tiles: compute starts early
    ("A", 6, 120, RA),
    ("B", 2, 128, RB),
    ("A", 2, 60, RA),       # small tail tiles: quick drain
)
CAP_A = sum(n * 128 * cg for rg, n, cg, _ in TILES if rg == "A")   # 122,880
CAP_B = sum(n * 128 * cg for rg, n, cg, _ in TILES if rg == "B")   # 32,768
NSLOT_A = CAP_A * RA        # 983,040
NSLOT_B = CAP_B * RB        # 131,072
NSLOT = NSLOT_A + NSLOT_B   # 1,114,112 slots per core

f16 = mybir.dt.float16


def build_nc() -> bass.Bass:
    nc = bacc.Bacc("TRN2")
    params = []
    for i, (rg, n, cg, r) in enumerate(TILES):
        params.append((
            nc.declare_dram_parameter(f"trow{i}", [n, 128, 12 * cg], f16, isOutput=False),
            nc.declare_dram_parameter(f"cent{i}", [n, 128, 3 * r * cg], f16, isOutput=False),
            nc.declare_dram_parameter(f"out{i}", [n, 128, 3 * r * cg], f16, isOutput=True),
        ))

    mul = mybir.AluOpType.mult
    add = mybir.AluOpType.add

    # per-class pools: the pool allocates bufs slots per distinct tag, so one
    # global deep pool over 4 tile-size classes overflows SBUF
    class_bufs = [min(n + 1, 5) for _, n, _, _ in TILES]

    with tile.TileContext(nc) as tc:
        with (
            tc.tile_pool(name="io0", bufs=class_bufs[0]) as p0,
            tc.tile_pool(name="io1", bufs=class_bufs[1]) as p1,
            tc.tile_pool(name="io2", bufs=class_bufs[2]) as p2,
            tc.tile_pool(name="io3", bufs=class_bufs[3]) as p3,
            tc.tile_pool(name="tmp", bufs=2) as tmp_pool,
        ):
            pools = [p0, p1, p2, p3]
            for i, (rg, n, cg, r) in enumerate(TILES):
                trow_in, cent_in, out = params[i]
                io_pool = pools[i]
                for t in range(n):
                    trow_t = io_pool.tile([128, 12 * cg], f16, tag=f"tr{i}")
                    nc.sync.dma_start(out=trow_t[:], in_=trow_in[t])
                    cent_t = io_pool.tile([128, 3 * r * cg], f16, tag=f"ce{i}")
                    nc.sync.dma_start(out=cent_t[:], in_=cent_in[t])

                    u = io_pool.tile([128, 3 * r * cg], f16, tag=f"u{i}")
                    tmp = tmp_pool.tile([128, 3 * r * cg], f16, tag=f"t{i}")

                    tr = trow_t[:].rearrange("p (dk c) -> p dk c", c=cg)
                    cr = cent_t[:].rearrange("p (j r c) -> p j r c", r=r, c=cg)
                    ur = u[:].rearrange("p (k r c) -> p k r c", r=r, c=cg)
                    tmr = tmp[:].rearrange("p (k r c) -> p k r c", r=r, c=cg)

                    def rows(d):  # trow planes d*3..d*3+3, bcast over r
                        return tr[:, 3 * d:3 * d + 3, :].unsqueeze(2).to_broadcast(
                            [128, 3, r, cg]
                        )

                    def xyz(j):  # cent plane j, bcast over k
                        return cr[:, j:j + 1, :, :].to_broadcast([128, 3, r, cg])

                    nc.vector.tensor_tensor(out=tmr, in0=xyz(0), in1=rows(1), op=mul)
                    nc.vector.tensor_tensor(out=ur, in0=rows(0), in1=tmr, op=add)
                    nc.vector.tensor_tensor(out=tmr, in0=xyz(1), in1=rows(2), op=mul)
                    nc.vector.tensor_tensor(out=ur, in0=ur, in1=tmr, op=add)
                    nc.vector.tensor_tensor(out=tmr, in0=xyz(2), in1=rows(3), op=mul)
                    nc.vector.tensor_tensor(out=ur, in0=ur, in1=tmr, op=add)

                    # stores ride the Activation engine's HWDGE queue so the
                    # next tiles' loads (Sync queue) are never stuck behind them
                    nc.scalar.dma_start(out=out[t], in_=u[:])
    nc.finalize()
    return nc


_NC_CACHE: dict = {}


def _get_nc():
    if TILES not in _NC_CACHE:
        _NC_CACHE[TILES] = build_nc()
    return _NC_CACHE[TILES]


def _prep_core(el, vox, coeffs16_c, cent16_full):
    """Build one core's device arrays from its (sorted) local element ids."""
    n = el.shape[0]
    L = np.bincount(el, minlength=EPC)
    q, m = L // RA, L % RA
    a_grp = q + (m >= 5)                     # R=8 groups per element
    b_grp = ((m >= 1) & (m <= 4)).astype(np.int64)   # 0/1 R=4 groups

    a_base = np.zeros(EPC, dtype=np.int64)
    np.cumsum(a_grp[:-1], out=a_base[1:])
    b_base = np.zeros(EPC, dtype=np.int64)
    np.cumsum(b_grp[:-1], out=b_base[1:])
    run_start = np.zeros(EPC, dtype=np.int64)
    np.cumsum(L[:-1], out=run_start[1:])

    rank = np.arange(n, dtype=np.int64) - run_start[el]
    athr = a_grp[el] * RA                    # slots this element owns in A
    in_a = rank < athr
    slot = np.where(
        in_a,
        a_base[el] * RA + rank,
        NSLOT_A + b_base[el] * RB + (rank - athr),
    )
    ok = np.where(in_a, slot < NSLOT_A, slot < NSLOT)

    trow_a = np.zeros((CAP_A, 12), dtype=np.float16)
    tot_a = int(a_grp.sum())
    rep = np.repeat(coeffs16_c, a_grp, axis=0)
    trow_a[:min(tot_a, CAP_A)] = rep[:CAP_A]
    trow_b = np.zeros((CAP_B, 12), dtype=np.float16)
    sel_b = coeffs16_c[b_grp.astype(bool)]
    trow_b[:min(sel_b.shape[0], CAP_B)] = sel_b[:CAP_B]

    cent_slot = np.zeros((NSLOT, 3), dtype=np.float16)
    cent_slot[slot[ok]] = cent16_full[vox[ok]]

    # slice group-major streams into per-tile-region planar arrays
    in_map = {}
    gA = gB = 0
    for i, (rg, nt, cg, r) in enumerate(TILES):
        ng = nt * 128 * cg
        if rg == "A":
            rows = trow_a[gA:gA + ng]
            cent = cent_slot[gA * RA:(gA + ng) * RA]
            gA += ng
        else:
            rows = trow_b[gB:gB + ng]
            cent = cent_slot[NSLOT_A + gB * RB:NSLOT_A + (gB + ng) * RB]
            gB += ng
        in_map[f"trow{i}"] = np.ascontiguousarray(
            rows.reshape(nt, 128, cg, 12).transpose(0, 1, 3, 2)
        ).reshape(nt, 128, 12 * cg)
        in_map[f"cent{i}"] = np.ascontiguousarray(
            cent.reshape(nt, 128, cg, r, 3).transpose(0, 1, 4, 3, 2)
        ).reshape(nt, 128, 3 * r * cg)

    return in_map, slot, ok


def _reassemble(results_c):
    """Concatenate per-tile-region outputs back to [NSLOT, 3] in slot order."""
    parts_a, parts_b = [], []
    for i, (rg, nt, cg, r) in enumerate(TILES):
        blk = results_c[f"out{i}"].reshape(nt, 128, 3, r, cg)
        flat = np.ascontiguousarray(blk.transpose(0, 1, 4, 3, 2)).reshape(-1, 3)
        (parts_a if rg == "A" else parts_b).append(flat)
    return np.concatenate(parts_a + parts_b, axis=0)


def kernel(all_coeffs, all_voxels_centroids, voxels_elements, _trace=False, **run_kwargs):
    nc = _get_nc()
    coeffs12 = np.asarray(all_coeffs, dtype=np.float32).reshape(N_ELEM, 12)
    coeffs16 = coeffs12.astype(np.float16)
    cent_full = np.asarray(all_voxels_centroids, dtype=np.float32)
    cent16 = cent_full.astype(np.float16)
    e_full = np.asarray(voxels_elements).astype(np.int64)

    order = np.argsort(e_full, kind="stable")
    es = e_full[order]
    bounds = np.searchsorted(es, np.arange(N_CORES + 1, dtype=np.int64) * EPC)

    in_maps, metas = [], []
    for c in range(N_CORES):
        lo, hi = int(bounds[c]), int(bounds[c + 1])
        vox = order[lo:hi]
        el = (es[lo:hi] - c * EPC).astype(np.int64)
        m, slot, ok = _prep_core(el, vox, coeffs16[c * EPC:(c + 1) * EPC], cent16)
        in_maps.append(m)
        metas.append((vox, slot, ok))

    res = run_bass_kernel_spmd(
        nc, in_maps, core_ids=list(range(N_CORES)), trace=_trace, **run_kwargs
    )

    full = np.empty((N_VOXELS, 3), dtype=np.float32)
    for c in range(N_CORES):
        vox, slot, ok = metas[c]
        u_slots = _reassemble(res.results[c])
        full[vox[ok]] = u_slots[slot[ok]].astype(np.float32)
        bad = ~ok
        if bad.any():
            vb = vox[bad]
            cf = coeffs12[e_full[vb]].reshape(-1, 4, 3)
            xyz = cent_full[vb]
            full[vb] = cf[:, 0] + np.einsum("nd,ndk->nk", xyz, cf[:, 1:4])
    if _trace:
        return full, res
    return full
